# revision 27
# baseline (speedup 1.0000x reference)
"""Trainium2 Bass kernel for DiagonalPositiveLinear:
    out[b, f] = input[b, f] * exp(log_weight[f])

Full-input contract: kernel() takes the full (8192, 4096) f32 input plus the
(4096,) f32 log_weight, shards rows across 8 NeuronCores (pure data parallel),
runs a raw-Bass kernel per core, and concatenates the row shards back.

Memory-bound: per core 16 MiB in + 16 MiB out over HBM. Raw Bass (not Tile)
because this toolchain's walrus allows only ONE sync wait per instruction; all
cross-engine sync is standalone wait_ge instructions and per-tile DMA
semaphores (a shared load semaphore would be unsound: the 16 per-DMA
increments from different tiles interleave).

Pipeline per core (N_TILES tiles of [128 partitions x T*4096 f32]):
  SP    : tile loads (HWDGE ring A), no waits
  Pool  : broadcast-load log_weight into 128 partitions (SWDGE)
  ACT   : exp(log_weight); then per tile: wait mul done -> store (HWDGE ring B)
  DVE   : wait exp; per tile: wait load -> in-place multiply by exp(w)
"""

import numpy as np

import concourse.bass as bass
from concourse import mybir
from concourse.bass_utils import run_bass_kernel_spmd

N_CORES = 8
ROWS, FEATS = 8192, 4096
SHARD_ROWS = ROWS // N_CORES          # 1024 rows per core
P = 128                               # SBUF partitions
T = 1                                 # rows-per-partition packed along free dim
TILE_FREE = T * FEATS
N_TILES = SHARD_ROWS // (P * T)

_F32 = mybir.dt.float32

_cached_nc = None

# Best-known configuration (updated from slope measurements in perf.py).
# All structural variants (tile shaping, split stores, dual store rings,
# gpsimd mul offload) measured equal within noise at ~100us/invocation —
# the per-core HBM bandwidth wall; [4,4] big-DMA tiling had the best medians.
BEST_CONFIG = dict(tiles=[4, 4], store_split=False, w_on_act=False)


def _build_bass(repeats=1, tiles=None, store_split=False, w_on_act=True,
                store_rings=1, mul_gp_units=0, mul_probe=None, load_rings=1):
    """repeats>1 builds a timing variant: the full load/mul/store pipeline is
    executed `repeats` times over the same data, so steady-state kernel time
    can be extracted as the slope w.r.t. repeats (amortizes dispatch cost).

    tiles: list of per-tile row-block counts (units of P=128 rows, i.e. each
    entry t_i gives a [128, t_i*FEATS] tile = t_i*2 MiB load/store DMA). Must
    sum to SHARD_ROWS//P == 8. Shaping the list tapers the pipeline: a small
    first tile lets stores start early, a small last tile shortens the drain.
    store_split: issue one store per FEATS-wide multiply slice instead of one
    per tile (finer store pipelining behind large tiles).
    w_on_act: broadcast-load log_weight via the ACT HWDGE ring (idle at start,
    ~0.6us fixed cost) instead of gpsimd SWDGE (~2us + slow Q7 descriptor
    generation for the 128-partition broadcast).
    """
    if tiles is None:
        tiles = [T] * (SHARD_ROWS // (P * T))
    assert sum(tiles) == SHARD_ROWS // P, tiles
    n_tiles = len(tiles)
    offs = np.cumsum([0] + list(tiles))  # row-block offset of each tile
    n_slices = sum(tiles)
    n_units = n_slices if store_split else n_tiles
    nc = bass.Bass()
    # 2D row-shard layout. Tile i covers rows [offs[i]*P, offs[i+1]*P);
    # within it partition p holds t_i CONSECUTIVE rows (p*t_i .. p*t_i+t_i)
    # concatenated along the free dim -> each partition is ONE contiguous
    # t_i*16KiB DRAM run (best DMA descriptor shape).
    x = nc.declare_dram_parameter("x", [SHARD_ROWS, FEATS], _F32, isOutput=False)
    w = nc.declare_dram_parameter("w", [FEATS], _F32, isOutput=False)
    y = nc.declare_dram_parameter("y", [SHARD_ROWS, FEATS], _F32, isOutput=True)

    with (
        nc.sbuf_tensor([P, n_slices * FEATS], _F32) as buf,
        nc.sbuf_tensor([P, FEATS], _F32) as wraw,
        nc.sbuf_tensor([P, FEATS], _F32) as wt,
        nc.semaphore("lw_sem") as lw_sem,      # log_weight broadcast load done
        nc.semaphore("wexp_sem") as wexp_sem,  # exp(w) computed
        nc.semaphore("mul_sem") as mul_sem,    # per-unit multiply done (in order)
        nc.semaphore("st_sem") as st_sem,      # store completions (total only)
        nc.Block() as block,
    ):
        ld_sems = [nc.alloc_semaphore(f"ld{i}") for i in range(n_tiles)]

        def tile_buf(i):
            return buf[:, offs[i] * FEATS : offs[i + 1] * FEATS]

        def tile_dram(handle, i):
            # rows [offs[i]*P, offs[i+1]*P) -> [P, t_i*FEATS], one contiguous
            # run per partition
            t_i = tiles[i]
            region = handle[offs[i] * P : offs[i + 1] * P, :]
            return region.rearrange("(p t) f -> p (t f)", p=P)

        def slice_dram(handle, i, j):
            # store AP for slice j of tile i: [P, FEATS], partition stride
            # t_i*FEATS, offset j*FEATS
            t_i = tiles[i]
            region = handle[offs[i] * P : offs[i + 1] * P, :]
            return region.rearrange("(p t) f -> p t f", p=P)[:, j, :]

        def emit_loads(eng, ring_idx):
            """Loads for tiles with i % load_rings == ring_idx."""
            for r in range(repeats):
                if r > 0:
                    # buffer slots reused across repeats: all repeat r-1
                    # stores must have drained (total-count semantics)
                    eng.wait_ge(st_sem, 16 * n_units * r)
                for i in range(n_tiles):
                    if i % load_rings != ring_idx:
                        continue
                    eng.dma_start(
                        out=tile_buf(i), in_=tile_dram(x, i)
                    ).then_inc(ld_sems[i], 16)

        @block.sync
        def _(sync):
            emit_loads(sync, 0)

        # multiply ownership: the last `mul_gp_units` TILES' multiplies run on
        # gpsimd (own completion sem) to take load off the DVE
        assert mul_gp_units == 0 or store_rings == 1
        gp_tiles = set(range(n_tiles - mul_gp_units, n_tiles))
        mulg_sem = nc.alloc_semaphore("mulg_sem") if gp_tiles else None

        # (unit u, tile i, dram-out AP, sbuf-in AP) per store DMA
        store_units = []
        u = 0
        for i in range(n_tiles):
            if store_split:
                for j in range(tiles[i]):
                    store_units.append(
                        (u, i, slice_dram(y, i, j),
                         tile_buf(i)[:, j * FEATS : (j + 1) * FEATS])
                    )
                    u += 1
            else:
                store_units.append((u, i, tile_dram(y, i), tile_buf(i)))
                u += 1

        # per-engine completion rank for each unit (sems inc in order within
        # each owner engine)
        owner_rank = {}
        dve_rank = gp_rank = 0
        for u, i, _, _ in store_units:
            if i in gp_tiles:
                gp_rank += 1
                owner_rank[u] = gp_rank
            else:
                dve_rank += 1
                owner_rank[u] = dve_rank
        dve_units_per_rep, gp_units_per_rep = dve_rank, gp_rank

        def unit_wait(eng, r, u, i):
            if i in gp_tiles:
                eng.wait_ge(mulg_sem, gp_units_per_rep * r + owner_rank[u])
            else:
                eng.wait_ge(mul_sem, dve_units_per_rep * r + owner_rank[u])

        def emit_stores(eng, ring_idx):
            """Stores for units with u % store_rings == ring_idx."""
            for r in range(repeats):
                for u, i, out_ap, in_ap in store_units:
                    if u % store_rings != ring_idx:
                        continue
                    unit_wait(eng, r, u, i)
                    eng.dma_start(out=out_ap, in_=in_ap).then_inc(st_sem, 16)
            eng.wait_ge(st_sem, 16 * n_units * repeats)

        def emit_muls(eng, my_tiles, sem):
            eng.wait_ge(wexp_sem, 1)
            for r in range(repeats):
                for i in my_tiles:
                    eng.wait_ge(ld_sems[i], 16 * (r + 1))
                    tb = tile_buf(i)
                    for j in range(tiles[i]):
                        sl = tb[:, j * FEATS : (j + 1) * FEATS]
                        if mul_probe == "scalar":
                            ins = eng.tensor_scalar_mul(sl, sl, 1.0)
                        else:
                            ins = eng.tensor_mul(sl, sl, wt[:])
                        if store_split:
                            ins.then_inc(sem, 1)
                    if not store_split:
                        ins.then_inc(sem, 1)

        if not w_on_act or store_rings > 1 or gp_tiles or load_rings > 1:

            @block.gpsimd
            def _(gpsimd):
                if not w_on_act:
                    gpsimd.dma_start(
                        out=wraw[:], in_=w[None, :].to_broadcast((P, FEATS))
                    ).then_inc(lw_sem, 16)
                if load_rings > 1:
                    emit_loads(gpsimd, 1)
                if gp_tiles:
                    emit_muls(gpsimd, sorted(gp_tiles), mulg_sem)
                if store_rings > 1:
                    emit_stores(gpsimd, 1)

        @block.vector
        def _(vector):
            emit_muls(vector, [i for i in range(n_tiles) if i not in gp_tiles],
                      mul_sem)

        @block.scalar
        def _(scalar):
            if w_on_act:
                scalar.dma_start(
                    out=wraw[:], in_=w[None, :].to_broadcast((P, FEATS))
                ).then_inc(lw_sem, 16)
            scalar.wait_ge(lw_sem, 16)
            scalar.activation(
                wt[:], wraw[:], mybir.ActivationFunctionType.Exp
            ).then_inc(wexp_sem, 1)
            emit_stores(scalar, 0)

    return nc


def _get_nc():
    global _cached_nc
    if _cached_nc is None:
        _cached_nc = _build_bass(**BEST_CONFIG)
    return _cached_nc


def _run(input, log_weight, trace=False, **spmd_kwargs):
    input = np.ascontiguousarray(np.asarray(input, dtype=np.float32))
    log_weight = np.ascontiguousarray(np.asarray(log_weight, dtype=np.float32))
    nc = _get_nc()
    in_maps = []
    for c in range(N_CORES):
        shard = input[c * SHARD_ROWS : (c + 1) * SHARD_ROWS]
        in_maps.append({"x": shard, "w": log_weight})
    res = run_bass_kernel_spmd(
        nc, in_maps, core_ids=list(range(N_CORES)), trace=trace, **spmd_kwargs
    )
    out = np.concatenate([r["y"] for r in res.results], axis=0)
    return out, res


def kernel(input, log_weight):
    out, _ = _run(input, log_weight, trace=False)
    return out
